# revision 19
# baseline (speedup 1.0000x reference)
"""CoPE attention kernel for Trainium2, SPMD over 8 NeuronCores.

Sharding: data-parallel over batch (2) x tensor-parallel over head groups (4):
core c handles batch c//4, heads [3*(c%4), 3*(c%4)+3).

Algorithm notes (per head, per 128-row q-tile):
  - logits = Q K^T (fp32r matmuls), gates = sigmoid(logits) via tanh identity
  - pos = reverse-cumsum(gates) clamped: computed as forward scan using the
    row total from the tanh pass's fused accumulator.
  - take_along_axis(A, floor/ceil(pos)) is reformulated gather-free:
    floor(pos) steps down by 0/1 along k (gates < 1), so crossings are
    scattered per-partition with local_scatter (GPSIMD), then hold/inject
    tensor_tensor_scans (DVE) expand table values along k.
  - softmax without max-subtraction (scores bounded), row sums via a ones
    column appended to V, probs transposed via DMA-xbar for the PV matmul.
"""
import sys
import types
import numpy as np

# -- walrus in this container rejects >1 sync wait on a CTRL instruction;
#    split the TileContext tail-drain waits onto standalone nops. --
def _install_tile_patch():
    import concourse.mybir as mybir
    from concourse import tile
    from concourse.vector_clock import ScopedClock

    if getattr(tile.TileContext, "_drain_patched", False):
        return

    def _drain_and_barrier_split(self, tick_clock, wait_clock):
        drain_inst = self.nc.sync.drain()
        wait_clock.add_sem_waits(
            drain_inst.ins, ScopedClock({None: tick_clock.global_clock})
        )
        si = drain_inst.ins.sync_info
        if si is not None and len(si.on_wait) > 1:
            waits = list(si.on_wait)
            si.on_wait = waits[:1]
            for i in range(1, len(waits)):
                nop = self.nc.sync.nop(nofuse=True)
                nsi = nop.ins.sync_info
                if nsi is None:
                    nop.ins.sync_info = mybir.SyncInfo(
                        on_wait=waits[i : i + 1], on_update=[]
                    )
                else:
                    nsi.on_wait = waits[i : i + 1]
        self.nc.all_engine_barrier()
        assert self.sems is not None
        popped = self.nc._tile_sem_poison_stack.pop()
        assert popped is self._sem_poison
        self.nc.clear_and_free_semaphores(list(self.sems.allocated().values()))
        self.nc.all_engine_barrier()

    tile.TileContext._drain_and_barrier = _drain_and_barrier_split
    tile.TileContext._drain_patched = True


B, T, H, DI, DK, DV = 2, 1152, 12, 768, 64, 64
SCALE = 1.0 / np.sqrt(DK)
NT = T // 128          # 9 q-tiles
NC_CHUNKS = DI // 128  # 6 contraction chunks
HPC = 3                # heads per core
N_CORES = 8
NEG = -1e30


def build(nc, T_=T, debug=False, dbg_h=1, dbg_r=8):
    """Build the SPMD per-core graph. T_ can be smaller for sim tests."""
    import concourse.mybir as mybir
    from concourse.tile import TileContext

    nt = T_ // 128
    f32 = mybir.dt.float32
    f32r = mybir.dt.float32r
    bf16 = mybir.dt.bfloat16
    f16 = mybir.dt.float16
    i16 = mybir.dt.int16
    Alu = mybir.AluOpType
    Act = mybir.ActivationFunctionType

    xT_ext = nc.declare_dram_parameter("xT", [DI, T_], f32, isOutput=False)
    w_ext = {}
    for name in ("wq", "wk", "wv", "wqs", "wks", "wvs"):
        w_ext[name] = nc.declare_dram_parameter(name, [HPC, DI, DK], f32, isOutput=False)
    cope_ext = nc.declare_dram_parameter("cope", [HPC, DK, T_], f32, isOutput=False)
    out_ext = nc.declare_dram_parameter("out", [HPC, T_, DV], f32, isOutput=True)
    dbg = {}
    if debug:
        for dn in ("d_pos", "d_scores", "d_d0", "d_d1", "d_w", "d_lg", "d_probs", "d_ptb", "d_atts"):
            dbg[dn] = nc.declare_dram_parameter(dn, [128, T_], f32, isOutput=True)
        import concourse.mybir as _mb
        for dn in ("d_fi", "d_ix1", "d_ix2", "d_psip", "d_v0"):
            dbg[dn] = nc.declare_dram_parameter(dn, [128, T_], _mb.dt.int16, isOutput=True)
        for dn in ("d_qt", "d_kt", "d_qt_early"):
            dbg[dn] = nc.declare_dram_parameter(dn, [64, T_], f32, isOutput=True)
        dbg["d_rs"] = nc.declare_dram_parameter("d_rs", [128, 2], f32, isOutput=True)

    SL = 64  # state length boundaries at 64 and T_-64
    seg_bounds = [(0, SL, True), (SL, T_ - SL, False), (T_ - SL, T_, True)]

    with TileContext(nc) as tc:
        with (
            tc.tile_pool(name="const", bufs=1) as cpool,
            tc.tile_pool(name="head", bufs=1) as hpool,
            tc.tile_pool(name="work", bufs=1) as wpool,
            tc.tile_pool(name="ps", bufs=1, space="PSUM") as ppool,
            tc.tile_pool(name="ps2", bufs=1, space="PSUM") as ppool2,
            tc.tile_pool(name="ps3", bufs=1, space="PSUM") as ppool3,
        ):
            # ---------- constants / inputs ----------
            xT_sb = cpool.tile([128, NC_CHUNKS, T_], f32)
            nc.sync.dma_start(xT_sb[:], xT_ext.rearrange("(c p) t -> p c t", p=128))
            w_sb = {}
            for name, ext in w_ext.items():
                wt = cpool.tile([128, HPC, NC_CHUNKS, DK], f32, tag=f"w_{name}")
                nc.sync.dma_start(wt[:], ext.rearrange("h (c p) k -> p h c k", p=128))
                w_sb[name] = wt
            cope_sb = cpool.tile([64, HPC, T_], f32)
            nc.sync.dma_start(cope_sb[:], cope_ext.rearrange("h d t -> d h t"))

            iota1 = cpool.tile([128, T_], i16)
            nc.gpsimd.iota(iota1[:], [[1, T_]], base=1, channel_multiplier=0)
            negs = cpool.tile([128, T_], f32)
            nc.vector.memset(negs[:], -1.0)
            zeros128 = cpool.tile([128, 128], f32)
            nc.vector.memset(zeros128[:], 0.0)
            dmask = cpool.tile([128, 128], f32)
            nc.gpsimd.affine_select(dmask[:], zeros128[:], [[-1, 128]], Alu.is_ge,
                                    fill=NEG, base=0, channel_multiplier=1)

            for h in range(HPC):
                # ---------- projections ----------
                qt_sb = hpool.tile([64, T_], f32, tag="qt_sb")
                kt_sb = hpool.tile([64, T_], f32, tag="kt_sb")
                for dst, wn, wsn in ((qt_sb, "wq", "wqs"), (kt_sb, "wk", "wks")):
                    for (s0, s1, is_state) in seg_bounds:
                        for o0 in range(s0, s1, 512):
                            o1 = min(o0 + 512, s1)
                            pj_ps = ppool3.tile([64, 512], f32, tag="pj_ps")
                            for c in range(NC_CHUNKS):
                                nc.tensor.matmul(
                                    pj_ps[:, 0:o1 - o0],
                                    w_sb[wsn if is_state else wn][:, h, c, :],
                                    xT_sb[:, c, o0:o1],
                                    start=(c == 0), stop=(c == NC_CHUNKS - 1))
                            nc.scalar.copy(dst[:, o0:o1], pj_ps[:, 0:o1 - o0])
                if debug and h == dbg_h:
                    nc.sync.dma_start(dbg["d_qt_early"][:], qt_sb[:])

                # V in [T, 64] layout, bf16, with ones column at 64
                vb_sb = hpool.tile([128, nt, DV + 1], bf16, tag="vb_sb")
                for r in range(nt):
                    v_ps_full = ppool2.tile([128, DV + 1], f32, tag="ps_small")
                    v_ps = v_ps_full[:, 0:DV]
                    t0 = r * 128
                    for (s0, s1, is_state) in seg_bounds:
                        lo, hi = max(s0, t0), min(s1, t0 + 128)
                        if lo >= hi:
                            continue
                        for c in range(NC_CHUNKS):
                            nc.tensor.matmul(
                                v_ps[lo - t0:hi - t0, :],
                                xT_sb[:, c, lo:hi],
                                w_sb["wvs" if is_state else "wv"][:, h, c, :],
                                start=(c == 0), stop=(c == NC_CHUNKS - 1))
                    nc.scalar.copy(vb_sb[:, r, 0:DV], v_ps[:])
                    nc.vector.memset(vb_sb[:, r, DV:DV + 1], 1.0)

                # ---------- q-tiles ----------
                for r in range(nt):
                    kw = 128 * (r + 1)
                    rr = slice(r * 128, (r + 1) * 128)

                    lg_ps = ppool.tile([128, T_], f32, tag="psA")
                    a_ps = ppool.tile([128, T_], f32, tag="psB")
                    for o0 in range(0, T_, 512):
                        o1 = min(o0 + 512, T_)
                        nc.tensor.matmul(lg_ps[:, o0:o1],
                                         qt_sb[:, rr],
                                         kt_sb[:, o0:o1],
                                         start=True, stop=True)
                    for o0 in range(0, T_, 512):
                        o1 = min(o0 + 512, T_)
                        nc.tensor.matmul(a_ps[:, o0:o1],
                                         qt_sb[:, rr],
                                         cope_sb[:, h, o0:o1],
                                         start=True, stop=True)

                    # gates via tanh(-x/2); row totals fused
                    ntn = wpool.tile([128, 1 + T_], f32, tag="ntn")
                    nc.vector.memset(ntn[:, 0:1], 0.0)
                    sumnt = wpool.tile([128, 1], f32, tag="sumnt")
                    nc.scalar.activation(ntn[:, 1:1 + T_], lg_ps[:], Act.Tanh,
                                         scale=-0.5, accum_out=sumnt[:])
                    # logits to SBUF (frees PSUM early; reread later for scores)
                    lg_sb = wpool.tile([128, T_], f32, tag="lg_sb")
                    nc.scalar.copy(lg_sb[:, 0:kw], lg_ps[:, 0:kw])

                    alpha1 = wpool.tile([128, 1], f32, tag="alpha1")
                    nc.vector.tensor_scalar(alpha1[:], sumnt[:], -1.0, float(T_ + 1),
                                            op0=Alu.mult, op1=Alu.add)
                    s2 = wpool.tile([128, T_], f32, tag="s2")
                    nc.vector.tensor_tensor_scan(s2[:, 0:kw], ntn[:, 0:kw],
                                                 negs[:, 0:kw], alpha1[:],
                                                 op0=Alu.add, op1=Alu.add)
                    pos_cl = wpool.tile([128, T_], f32, tag="pos_cl")
                    nc.vector.tensor_scalar(pos_cl[:, 0:kw], s2[:, 0:kw], 0.5,
                                            float(T - 1), op0=Alu.mult, op1=Alu.min)
                    fi = wpool.tile([128, 1 + T_], i16, tag="fi")
                    nc.vector.memset(fi[:, 0:1], 32767)
                    nc.vector.tensor_scalar(fi[:, 1:1 + kw], pos_cl[:, 0:kw],
                                            -0.4999999, None, op0=Alu.add)
                    wfrac = wpool.tile([128, T_], f16, tag="wfrac")
                    nc.vector.tensor_tensor(wfrac[:, 0:kw], pos_cl[:, 0:kw],
                                            fi[:, 1:1 + kw], Alu.subtract)
                    fp1 = wpool.tile([128, T_], i16, tag="fp1")
                    nc.vector.tensor_scalar(fp1[:, 0:kw], fi[:, 1:1 + kw], 1, None,
                                            op0=Alu.add)
                    mbf = wpool.tile([128, T_], f16, tag="mbf")
                    nc.vector.tensor_tensor(mbf[:, 0:kw], fi[:, 1:1 + kw],
                                            fi[:, 0:kw], Alu.is_lt)
                    nmbf = wpool.tile([128, T_], f16, tag="nmbf")
                    nc.vector.tensor_scalar(nmbf[:, 0:kw], mbf[:, 0:kw], -1.0, 1.0,
                                            op0=Alu.mult, op1=Alu.add)
                    ix1a = wpool.tile([128, T_], i16, tag="ix1a")
                    nc.vector.tensor_tensor(ix1a[:, 0:kw], fp1[:, 0:kw],
                                            mbf[:, 0:kw], Alu.mult)
                    ix1 = wpool.tile([128, T_], i16, tag="ix1")
                    nc.vector.tensor_scalar(ix1[:, 0:kw], ix1a[:, 0:kw], -1.0, None,
                                            op0=Alu.add)

                    psip = wpool.tile([128, T_], i16, tag="psip")
                    nc.gpsimd.local_scatter(psip[:], iota1[:, 0:kw], ix1[:, 0:kw],
                                            channels=128, num_elems=T_, num_idxs=kw)
                    TW = min(768, T_)
                    ix2 = wpool.tile([128, T_], i16, tag="ix2")
                    nc.vector.tensor_scalar(ix2[:, 0:TW], psip[:, 0:TW], -1.0, None,
                                            op0=Alu.add)

                    TW = min(768, T_)
                    abf = wpool.tile([128, T_ + 2], f16, tag="abf")
                    nc.scalar.copy(abf[:, 0:TW + 2], a_ps[:, 0:TW + 2])

                    v0 = wpool.tile([128, T_], f16, tag="v0")
                    nc.gpsimd.local_scatter(v0[:, 0:kw].bitcast(i16),
                                            abf[:, 0:TW].bitcast(i16), ix2[:, 0:TW],
                                            channels=128, num_elems=kw, num_idxs=TW)
                    abf1 = wpool.tile([128, T_], f16, tag="abf1")
                    nc.vector.tensor_copy(abf1[:, 0:TW], abf[:, 1:TW + 1])
                    v1 = wpool.tile([128, T_], f16, tag="v1")
                    nc.gpsimd.local_scatter(v1[:, 0:kw].bitcast(i16),
                                            abf1[:, 0:TW].bitcast(i16), ix2[:, 0:TW],
                                            channels=128, num_elems=kw, num_idxs=TW)

                    d0 = wpool.tile([128, T_], f16, tag="d0")
                    nc.vector.tensor_tensor_scan(d0[:, 0:kw], nmbf[:, 0:kw],
                                                 v0[:, 0:kw], 0.0,
                                                 op0=Alu.mult, op1=Alu.add)
                    d1 = wpool.tile([128, T_], f16, tag="d1")
                    nc.vector.tensor_tensor_scan(d1[:, 0:kw], nmbf[:, 0:kw],
                                                 v1[:, 0:kw], 0.0,
                                                 op0=Alu.mult, op1=Alu.add)
                    dd = wpool.tile([128, T_], f16, tag="dd")
                    nc.vector.tensor_tensor(dd[:, 0:kw], d1[:, 0:kw], d0[:, 0:kw],
                                            Alu.subtract)
                    t2 = wpool.tile([128, T_], f16, tag="t2")
                    nc.vector.tensor_tensor(t2[:, 0:kw], wfrac[:, 0:kw], dd[:, 0:kw],
                                            Alu.mult)
                    s1t = wpool.tile([128, T_], f32, tag="s1t")
                    nc.vector.scalar_tensor_tensor(s1t[:, 0:kw], lg_sb[:, 0:kw],
                                                   float(SCALE), d0[:, 0:kw],
                                                   op0=Alu.mult, op1=Alu.add)
                    scores = wpool.tile([128, T_], f32, tag="scores")
                    nc.vector.tensor_tensor(scores[:, 0:kw], s1t[:, 0:kw],
                                            t2[:, 0:kw], Alu.add)
                    # causal mask on diagonal block
                    nc.vector.tensor_tensor(scores[:, r * 128:kw],
                                            scores[:, r * 128:kw], dmask[:],
                                            Alu.add)
                    probs = wpool.tile([128, T_], bf16, tag="probs")
                    nc.scalar.activation(probs[:, 0:kw], scores[:, 0:kw], Act.Exp)

                    ptb = wpool.tile([128, nt, 128], bf16, tag="ptb")
                    for c in range(r + 1):
                        eng = nc.sync if c % 2 == 0 else nc.scalar
                        eng.dma_start_transpose(ptb[:, c, :],
                                                probs[:, c * 128:(c + 1) * 128])
                    att_ps = ppool2.tile([128, DV + 1], f32, tag="ps_small")
                    for c in range(r + 1):
                        nc.tensor.matmul(att_ps[:], ptb[:, c, :], vb_sb[:, c, :],
                                         start=(c == 0), stop=(c == r))
                    rcp = wpool.tile([128, 1], f32, tag="rcp")
                    nc.vector.reciprocal(rcp[:], att_ps[:, DV:DV + 1])
                    atts = wpool.tile([128, DV], f32, tag="atts")
                    nc.vector.tensor_scalar(atts[:], att_ps[:, 0:DV], rcp[:], None,
                                            op0=Alu.mult)
                    nc.sync.dma_start(out_ext[h, rr, :], atts[:])
                    if debug and h == dbg_h and r == dbg_r:
                        nc.sync.dma_start(dbg["d_pos"][:, 0:kw], pos_cl[:, 0:kw])
                        nc.sync.dma_start(dbg["d_scores"][:, 0:kw], scores[:, 0:kw])
                        dcp0 = wpool.tile([128, T_], f32, tag="dcp0")
                        nc.vector.tensor_copy(dcp0[:, 0:kw], d0[:, 0:kw])
                        nc.sync.dma_start(dbg["d_d0"][:, 0:kw], dcp0[:, 0:kw])
                        dcp1 = wpool.tile([128, T_], f32, tag="dcp1")
                        nc.vector.tensor_copy(dcp1[:, 0:kw], d1[:, 0:kw])
                        nc.sync.dma_start(dbg["d_d1"][:, 0:kw], dcp1[:, 0:kw])
                        dcpw = wpool.tile([128, T_], f32, tag="dcpw")
                        nc.vector.tensor_copy(dcpw[:, 0:kw], wfrac[:, 0:kw])
                        nc.sync.dma_start(dbg["d_w"][:, 0:kw], dcpw[:, 0:kw])
                        nc.sync.dma_start(dbg["d_lg"][:, 0:kw], lg_sb[:, 0:kw])
                        nc.sync.dma_start(dbg["d_qt"][:], qt_sb[:])
                        nc.sync.dma_start(dbg["d_kt"][:], kt_sb[:])
                        dcrs = wpool.tile([128, 2], f32, tag="dcrs")
                        nc.vector.tensor_copy(dcrs[:, 0:1], att_ps[:, DV:DV+1])
                        nc.vector.tensor_copy(dcrs[:, 1:2], rcp[:])
                        nc.sync.dma_start(dbg["d_rs"][:], dcrs[:])
                        dcpp = wpool.tile([128, T_], f32, tag="dcpp")
                        nc.vector.tensor_copy(dcpp[:, 0:kw], probs[:, 0:kw])
                        nc.sync.dma_start(dbg["d_probs"][:, 0:kw], dcpp[:, 0:kw])
                        nc.vector.tensor_copy(dcpp[:, 0:kw], ptb[:, 0:r + 1, :].rearrange("p c k -> p (c k)"))
                        nc.sync.dma_start(dbg["d_ptb"][:, 0:kw], dcpp[:, 0:kw])
                        nc.sync.dma_start(dbg["d_atts"][:, 0:DV], atts[:])
                        nc.sync.dma_start(dbg["d_fi"][:, 0:kw], fi[:, 1:1 + kw])
                        nc.sync.dma_start(dbg["d_ix1"][:, 0:kw], ix1[:, 0:kw])
                        nc.sync.dma_start(dbg["d_ix2"][:], ix2[:])
                        nc.sync.dma_start(dbg["d_psip"][:], psip[:])
                        nc.sync.dma_start(dbg["d_v0"][:, 0:kw], v0[:, 0:kw].bitcast(_mb.dt.int16))
    return nc


_CACHE = {}


def _get_compiled():
    if "nc" not in _CACHE:
        _install_tile_patch()
        from concourse import bacc
        nc = bacc.Bacc()
        build(nc)
        nc.compile()
        _CACHE["nc"] = nc
    return _CACHE["nc"]


def kernel(x, w_q, w_k, w_v, w_q_state, w_k_state, w_v_state, cope_emb):
    _install_tile_patch()
    from concourse.bass_utils import run_bass_kernel_spmd

    nc = _get_compiled()
    x = np.ascontiguousarray(np.asarray(x, dtype=np.float32))
    cope = np.asarray(cope_emb, dtype=np.float32)[0]  # (H, DK, T)
    ws = {
        "wq": np.asarray(w_q, np.float32),
        "wk": np.asarray(w_k, np.float32),
        "wv": np.asarray(w_v, np.float32),
        "wqs": np.asarray(w_q_state, np.float32),
        "wks": np.asarray(w_k_state, np.float32),
        "wvs": np.asarray(w_v_state, np.float32),
    }
    in_maps = []
    for c in range(N_CORES):
        b = c // 4
        h0 = HPC * (c % 4)
        m = {"xT": np.ascontiguousarray(x[b].T)}
        for name, w in ws.items():
            m[name] = np.ascontiguousarray(w[h0:h0 + HPC])
        m["cope"] = np.ascontiguousarray(cope[h0:h0 + HPC])
        in_maps.append(m)
    res = run_bass_kernel_spmd(nc, in_maps, core_ids=list(range(N_CORES)))
    out = np.zeros((B, H, T, DV), np.float32)
    for c in range(N_CORES):
        b = c // 4
        h0 = HPC * (c % 4)
        out[b, h0:h0 + HPC] = res.results[c]["out"]
    return out
